# revision 20
# baseline (speedup 1.0000x reference)
# Block-circulant linear kernel for Trainium2 (Bass/Tile), 8-core SPMD.
#
# y[b, 16m+p] = sum_{n,q} blocks[(m-n)%512, p, q] * x[b, 16n+q]
#
# Strategy: shard the output block axis m across 8 cores (64 block-rows each).
# Per core, store a doubled+shifted "BIGQ" layout of blocks in SBUF:
#     BIGQ[(ni,q), u*16+p] = blocks[(m0 + u - ni) % 512, p, q]
# so that EVERY 128x128 weight tile of the implied 8192x8192 circulant matrix
# is a contiguous 128-column slice of BIGQ (the circulant gather becomes pure
# addressing). All (m_tile t, n_chunk c) pairs with the same diagonal offset
# d = t - c share one stationary tile, so the whole per-core compute is 71
# accumulating matmuls into a single PSUM bank [128 mp, 8 t x 32 b].
#
# The xt layout is reversed (c' = 63 - c) and the psum t axis flipped
# (t' = 7 - t) so both the weight stream (BIGQ u ascending) and the moving
# stream (xt c' ascending) are consumed in DMA arrival order.
import numpy as np

B = 32
NB = 512          # number of 16x16 blocks
NCORES = 8
MBLK = NB // NCORES   # 64 output block-rows per core
W = 576               # BIGQ window width (in u units of 16 columns)
ND = 71               # diagonal offsets d in [-63, 7]

# matmul operand dtype: "float32" (exact, 4 cyc/row), "float32r" (1 cyc/row
# at N>=256), "bfloat16" (1 cyc/row + fast weight load, ~1e-3 error).
# A "wt:mv" pair gives the weight (bigq) and moving (xt) dtypes separately.
DTYPE = "float32"

_cached = {}
_last_results = None  # BassKernelResults of the most recent run (for profiling)


def _np_dtype(name):
    if name == "bfloat16":
        import ml_dtypes

        return ml_dtypes.bfloat16
    if name == "float16":
        return np.float16
    return np.float32


def _split_dt(dt_name):
    """'wt:mv' -> (weight dtype, moving dtype); single name -> same both."""
    if ":" in dt_name:
        wt, mv = dt_name.split(":")
        return wt, mv
    return dt_name, dt_name


def _build_program(dt_name):
    import concourse.bacc as bacc
    import concourse.mybir as mybir
    import concourse.tile as tile

    wt_name, mv_name = _split_dt(dt_name)
    wdt = getattr(mybir.dt, wt_name)
    mdt = getattr(mybir.dt, mv_name)
    f32 = mybir.dt.float32

    # Bacc (not plain Bass): its compile() pipeline splits multi-wait
    # instructions into EventSemaphore preludes (HW allows 1 wait/inst).
    nc = bacc.Bacc("TRN2", target_bir_lowering=False, debug=False, num_devices=NCORES)
    xt_d = nc.declare_dram_parameter("xt", [128, 2048], mdt, isOutput=False)
    bq_d = nc.declare_dram_parameter("bigq", [128, W * 16], wdt, isOutput=False)
    out_d = nc.declare_dram_parameter("out", [128, 256], f32, isOutput=True)

    NCH = 8
    csz = (W * 16) // NCH  # 1152 bigq cols per chunk

    with tile.TileContext(nc) as tc:
        with (
            tc.tile_pool(name="data", bufs=1) as data_pool,
            tc.tile_pool(name="psum", bufs=1, space="PSUM") as psum_pool,
        ):
            xt = data_pool.tile([128, 2048], mdt)
            bq = data_pool.tile([128, W * 16], wdt)
            out_sb = data_pool.tile([128, 256], f32)
            warm_sb = data_pool.tile([128, 256], f32)
            acc = psum_pool.tile([128, 256], f32)
            warm_ps = psum_pool.tile([128, 256], f32)

            # interleave the streams in consumption order: first xt half +
            # first bigq chunks feed the earliest matmuls. Alternate the two
            # HWDGE issue engines (sync=SP, scalar=ACT) so descriptor
            # generation isn't serialized on one sequencer.
            eng = [nc.sync, nc.scalar]
            eng[0].dma_start(xt[:, 0:1024], xt_d[:, 0:1024])
            for ci in range(NCH // 2):
                eng[(ci + 1) % 2].dma_start(
                    bq[:, ci * csz:(ci + 1) * csz], bq_d[:, ci * csz:(ci + 1) * csz]
                )
            eng[1].dma_start(xt[:, 1024:2048], xt_d[:, 1024:2048])
            for ci in range(NCH // 2, NCH):
                eng[ci % 2].dma_start(
                    bq[:, ci * csz:(ci + 1) * csz], bq_d[:, ci * csz:(ci + 1) * csz]
                )

            # PE warm-up while DMA streams in: >=3.4us of dummy matmul work
            # flips the HAM clock gate to 2.4GHz before the real stream. Each
            # fp32 N=256 matmul lowers to 2 instructions of ~427ns; 4 logical
            # matmuls ~= 3.4us, ending right around first-chunk arrival.
            nwarm = 4
            nc.gpsimd.memset(warm_sb[:], 0.0)
            for wi in range(nwarm):
                nc.tensor.matmul(
                    warm_ps[:], warm_sb[:, 0:128], warm_sb[:],
                    start=(wi == 0), stop=(wi == nwarm - 1),
                )

            # d = t - c diagonal; stationary tile = BIGQ columns [16*u0, 16*u0+128)
            # with u0 = 8*i + 8 for i = 0..70 (d = i - 63).
            for i in range(ND):
                d = i - 63
                u0 = 8 * i + 8
                t_lo = max(0, d)
                t_hi = min(7, 63 + d)
                nt = t_hi - t_lo + 1
                tp_lo = 7 - t_hi           # flipped psum tile index
                cp_lo = 63 + d - t_hi      # reversed xt chunk index
                nc.tensor.matmul(
                    acc[:, 32 * tp_lo: 32 * (tp_lo + nt)],
                    bq[:, 16 * u0: 16 * u0 + 128],
                    xt[:, 32 * cp_lo: 32 * (cp_lo + nt)],
                    start=(i == 0),   # clears the whole PSUM bank
                    stop=(i == ND - 1),
                )

            nc.vector.tensor_copy(out_sb[:], acc[:])
            nc.sync.dma_start(out_d[:], out_sb[:])
    nc.compile()
    return nc


def _get_program(dt_name):
    if dt_name not in _cached:
        _cached[dt_name] = _build_program(dt_name)
    return _cached[dt_name]


def _prep_inputs(x, blocks, dt_name):
    """Host-side layout prep (pure numpy reshuffles of the small inputs)."""
    wt_name, mv_name = _split_dt(dt_name)
    np_w, np_m = _np_dtype(wt_name), _np_dtype(mv_name)
    x = np.ascontiguousarray(np.asarray(x), dtype=np.float32)
    blocks = np.ascontiguousarray(np.asarray(blocks), dtype=np.float32)
    # xt[(ni*16+q), c*32+b] = x[b, 128c + 16ni + q], then reverse c (c'=63-c)
    xt = x.T.reshape(64, 128, 32).transpose(1, 0, 2)[:, ::-1, :].reshape(128, 2048)
    xt = np.ascontiguousarray(xt.astype(np_m))
    u = np.arange(W)
    ni = np.arange(8)
    in_maps = []
    for k in range(NCORES):
        m0 = k * MBLK
        idx = (m0 + u[None, :] - ni[:, None]) % NB        # [8, W]
        bigq = blocks[idx]                                 # [8, W, p, q]
        bigq = bigq.transpose(0, 3, 1, 2).reshape(128, W * 16)  # [(ni,q), (u,p)]
        in_maps.append({"xt": xt, "bigq": np.ascontiguousarray(bigq.astype(np_w))})
    return in_maps


def _assemble(results):
    y = np.empty((B, NB * 16), dtype=np.float32)
    for k in range(NCORES):
        o = np.asarray(results[k]["out"])  # [128 (mi,p), 256 (t',b)], t = 7-t'
        y[:, 1024 * k: 1024 * (k + 1)] = (
            o.reshape(128, 8, 32)[:, ::-1, :].transpose(2, 1, 0).reshape(32, 1024)
        )
    return y


def kernel(x, blocks):
    global _last_results
    from concourse.bass_utils import run_bass_kernel_spmd

    nc = _get_program(DTYPE)
    in_maps = _prep_inputs(x, blocks, DTYPE)
    res = run_bass_kernel_spmd(nc, in_maps, list(range(NCORES)))
    _last_results = res
    return _assemble(res.results)


# revision 27
# speedup vs baseline: 1.3790x; 1.3790x over previous
# Block-circulant linear kernel for Trainium2 (Bass/Tile), 8-core SPMD.
#
# y[b, 16m+p] = sum_{n,q} blocks[(m-n)%512, p, q] * x[b, 16n+q]
#
# Strategy: shard the output block axis m across 8 cores (64 block-rows each).
# Per core, store a doubled+shifted "BIGQ" layout of blocks in SBUF:
#     BIGQ[(ni,q), u*16+p] = blocks[(m0 + u - ni) % 512, p, q]
# so that EVERY 128x128 weight tile of the implied 8192x8192 circulant matrix
# is a contiguous 128-column slice of BIGQ (the circulant gather becomes pure
# addressing). All (m_tile t, n_chunk c) pairs with the same diagonal offset
# d = t - c share one stationary tile, so the whole per-core compute is 71
# accumulating matmuls into a single PSUM bank [128 mp, 8 t x 32 b].
#
# The xt layout is reversed (c' = 63 - c) and the psum t axis flipped
# (t' = 7 - t) so both the weight stream (BIGQ u ascending) and the moving
# stream (xt c' ascending) are consumed in DMA arrival order.
import numpy as np

B = 32
NB = 512          # number of 16x16 blocks
NCORES = 8
MBLK = NB // NCORES   # 64 output block-rows per core
W = 576               # BIGQ window width (in u units of 16 columns)
ND = 71               # diagonal offsets d in [-63, 7]

# matmul operand dtype: "float32" (exact, 4 cyc/row), "float32r" (1 cyc/row
# at N>=256, tf32-class ~1.3e-4 error), "float16" (1 cyc/row + fast weight
# load, ~2.5e-4 error), "split16" (two-level fp16 split, fp32-class ~3e-7
# error at ~44us). A "wt:mv" pair gives weight/moving dtypes separately.
# Measured (8-core, HW): float16 25us / float32r 32us / split16 44us /
# float32 50us.
DTYPE = "float16"

_cached = {}
_last_results = None  # BassKernelResults of the most recent run (for profiling)


def _np_dtype(name):
    if name == "bfloat16":
        import ml_dtypes

        return ml_dtypes.bfloat16
    if name == "float16":
        return np.float16
    return np.float32


def _split_dt(dt_name):
    """'wt:mv' -> (weight dtype, moving dtype); single name -> same both."""
    if ":" in dt_name:
        wt, mv = dt_name.split(":")
        return wt, mv
    return dt_name, dt_name


def _build_split16(nterm=3):
    """fp16 split-precision: A = A_hi + 2^-11 A_lo', x = x_hi + 2^-11 x_lo'
    (lo parts stored pre-scaled by 2^11 so they stay fp16-normal). Terms
    hi*hi -> acc_m; hi*lo + lo*hi -> acc_l (scale 2^-11); products are exact
    in fp32 PSUM, so the result is fp32-class accurate at fp16 speeds."""
    import concourse.bacc as bacc
    import concourse.mybir as mybir
    import concourse.tile as tile

    f16 = mybir.dt.float16
    f32 = mybir.dt.float32
    nc = bacc.Bacc("TRN2", target_bir_lowering=False, debug=False, num_devices=NCORES)
    xth_d = nc.declare_dram_parameter("xth", [128, 2048], f16, isOutput=False)
    xtl_d = nc.declare_dram_parameter("xtl", [128, 2048], f16, isOutput=False)
    bqh_d = nc.declare_dram_parameter("bqh", [128, W * 16], f16, isOutput=False)
    bql_d = nc.declare_dram_parameter("bql", [128, W * 16], f16, isOutput=False)
    out_d = nc.declare_dram_parameter("out", [128, 256], f32, isOutput=True)

    csz = (W * 16) // 4  # 2304 bigq cols per chunk

    with tile.TileContext(nc) as tc:
        with (
            tc.tile_pool(name="data", bufs=1) as data_pool,
            tc.tile_pool(name="psum", bufs=1, space="PSUM") as psum_pool,
        ):
            xth = data_pool.tile([128, 2048], f16)
            xtl = data_pool.tile([128, 2048], f16)
            bqh = data_pool.tile([128, W * 16], f16)
            bql = data_pool.tile([128, W * 16], f16)
            out_sb = data_pool.tile([128, 256], f32)
            tmp_sb = data_pool.tile([128, 256], f32)
            warm_sb = data_pool.tile([128, 256], f32)
            acc_m = psum_pool.tile([128, 256], f32)
            acc_l = psum_pool.tile([128, 256], f32)
            warm_ps = psum_pool.tile([128, 256], f32)

            eng = [nc.sync, nc.scalar]
            order = [
                (xth, xth_d, 0, 1024), (xtl, xtl_d, 0, 1024),
                (bqh, bqh_d, 0, csz), (bql, bql_d, 0, csz),
                (bqh, bqh_d, csz, 2 * csz), (bql, bql_d, csz, 2 * csz),
                (xth, xth_d, 1024, 2048), (xtl, xtl_d, 1024, 2048),
                (bqh, bqh_d, 2 * csz, 3 * csz), (bql, bql_d, 2 * csz, 3 * csz),
                (bqh, bqh_d, 3 * csz, 4 * csz), (bql, bql_d, 3 * csz, 4 * csz),
            ]
            for k, (tile_, dram_, lo, hi) in enumerate(order):
                eng[k % 2].dma_start(tile_[:, lo:hi], dram_[:, lo:hi])

            nwarm = 4
            nc.gpsimd.memset(warm_sb[:], 0.0)
            for wi in range(nwarm):
                nc.tensor.matmul(
                    warm_ps[:], warm_sb[:, 0:128], warm_sb[:],
                    start=(wi == 0), stop=(wi == nwarm - 1),
                )

            for i in range(ND):
                d = i - 63
                u0 = 8 * i + 8
                t_lo = max(0, d)
                t_hi = min(7, 63 + d)
                nt = t_hi - t_lo + 1
                tp_lo = 7 - t_hi
                cp_lo = 63 + d - t_hi
                ops = 32 * tp_lo, 32 * (tp_lo + nt)       # psum col slice
                xs = 32 * cp_lo, 32 * (cp_lo + nt)        # xt col slice
                ws = 16 * u0, 16 * u0 + 128               # weight col slice
                last = i == ND - 1
                nc.tensor.matmul(
                    acc_m[:, ops[0]:ops[1]], bqh[:, ws[0]:ws[1]],
                    xth[:, xs[0]:xs[1]], start=(i == 0), stop=last,
                    skip_group_check=True,
                )
                nc.tensor.matmul(
                    acc_l[:, ops[0]:ops[1]], bqh[:, ws[0]:ws[1]],
                    xtl[:, xs[0]:xs[1]], start=(i == 0), stop=False,
                    skip_group_check=True,
                )
                nc.tensor.matmul(
                    acc_l[:, ops[0]:ops[1]], bql[:, ws[0]:ws[1]],
                    xth[:, xs[0]:xs[1]], start=False, stop=last,
                    skip_group_check=True,
                )

            nc.vector.tensor_scalar_mul(tmp_sb[:], acc_l[:], float(2.0 ** -11))
            nc.vector.tensor_add(out_sb[:], acc_m[:], tmp_sb[:])
            nc.sync.dma_start(out_d[:], out_sb[:])
    nc.compile()
    return nc


def _build_program(dt_name):
    import concourse.bacc as bacc
    import concourse.mybir as mybir
    import concourse.tile as tile

    if dt_name == "split16":
        return _build_split16()

    wt_name, mv_name = _split_dt(dt_name)
    wdt = getattr(mybir.dt, wt_name)
    mdt = getattr(mybir.dt, mv_name)
    f32 = mybir.dt.float32

    # Bacc (not plain Bass): its compile() pipeline splits multi-wait
    # instructions into EventSemaphore preludes (HW allows 1 wait/inst).
    nc = bacc.Bacc("TRN2", target_bir_lowering=False, debug=False, num_devices=NCORES)
    xt_d = nc.declare_dram_parameter("xt", [128, 2048], mdt, isOutput=False)
    bq_d = nc.declare_dram_parameter("bigq", [128, W * 16], wdt, isOutput=False)
    out_d = nc.declare_dram_parameter("out", [128, 256], f32, isOutput=True)

    two_byte = False  # measured: chunking variants are within run-to-run noise
    NCH = 8
    csz = (W * 16) // NCH  # bigq cols per chunk (multiple of 128)

    with tile.TileContext(nc) as tc:
        with (
            tc.tile_pool(name="data", bufs=1) as data_pool,
            tc.tile_pool(name="psum", bufs=1, space="PSUM") as psum_pool,
        ):
            xt = data_pool.tile([128, 2048], mdt)
            bq = data_pool.tile([128, W * 16], wdt)
            out_sb = data_pool.tile([128, 256], f32)
            warm_sb = data_pool.tile([128, 256], f32)
            acc = psum_pool.tile([128, 256], f32)
            warm_ps = psum_pool.tile([128, 256], f32)

            # interleave the streams in consumption order: first xt half +
            # first bigq chunks feed the earliest matmuls. Alternate the two
            # HWDGE issue engines (sync=SP, scalar=ACT) so descriptor
            # generation isn't serialized on one sequencer.
            eng = [nc.sync, nc.scalar]
            if two_byte:
                # xt is only 0.5MB at 2 bytes: one DMA, then 6 bigq chunks.
                eng[0].dma_start(xt[:], xt_d[:])
                for ci in range(NCH):
                    eng[(ci + 1) % 2].dma_start(
                        bq[:, ci * csz:(ci + 1) * csz],
                        bq_d[:, ci * csz:(ci + 1) * csz],
                    )
            else:
                eng[0].dma_start(xt[:, 0:1024], xt_d[:, 0:1024])
                for ci in range(NCH // 2):
                    eng[(ci + 1) % 2].dma_start(
                        bq[:, ci * csz:(ci + 1) * csz],
                        bq_d[:, ci * csz:(ci + 1) * csz],
                    )
                eng[1].dma_start(xt[:, 1024:2048], xt_d[:, 1024:2048])
                for ci in range(NCH // 2, NCH):
                    eng[ci % 2].dma_start(
                        bq[:, ci * csz:(ci + 1) * csz],
                        bq_d[:, ci * csz:(ci + 1) * csz],
                    )

            # PE warm-up while DMA streams in: >=3.4us of dummy matmul work
            # flips the HAM clock gate to 2.4GHz before the real stream. Each
            # fp32 N=256 matmul lowers to 2 instructions of ~427ns; 4 logical
            # matmuls ~= 3.4us, ending right around first-chunk arrival.
            # Only worth it for 2-byte streams: the 4-byte variants are
            # DMA-bound, where a warm PE just races ahead into chunk waits and
            # the idle gaps re-throttle the clock anyway.
            nwarm = 4 if mv_name in ("float16", "bfloat16") else 0
            if nwarm:
                nc.gpsimd.memset(warm_sb[:], 0.0)
            for wi in range(nwarm):
                nc.tensor.matmul(
                    warm_ps[:], warm_sb[:, 0:128], warm_sb[:],
                    start=(wi == 0), stop=(wi == nwarm - 1),
                )

            # d = t - c diagonal; stationary tile = BIGQ columns [16*u0, 16*u0+128)
            # with u0 = 8*i + 8 for i = 0..70 (d = i - 63).
            for i in range(ND):
                d = i - 63
                u0 = 8 * i + 8
                t_lo = max(0, d)
                t_hi = min(7, 63 + d)
                nt = t_hi - t_lo + 1
                tp_lo = 7 - t_hi           # flipped psum tile index
                cp_lo = 63 + d - t_hi      # reversed xt chunk index
                nc.tensor.matmul(
                    acc[:, 32 * tp_lo: 32 * (tp_lo + nt)],
                    bq[:, 16 * u0: 16 * u0 + 128],
                    xt[:, 32 * cp_lo: 32 * (cp_lo + nt)],
                    start=(i == 0),   # clears the whole PSUM bank
                    stop=(i == ND - 1),
                )

            nc.vector.tensor_copy(out_sb[:], acc[:])
            nc.sync.dma_start(out_d[:], out_sb[:])
    nc.compile()
    return nc


def _get_program(dt_name):
    if dt_name not in _cached:
        _cached[dt_name] = _build_program(dt_name)
    return _cached[dt_name]


def _hi_lo16(a):
    """fp16 two-level split: a ~= hi + 2^-11 * lo (lo pre-scaled by 2^11)."""
    hi = a.astype(np.float16)
    lo = ((a - hi.astype(np.float32)) * 2048.0).astype(np.float16)
    return hi, lo


def _prep_inputs(x, blocks, dt_name):
    """Host-side layout prep (pure numpy reshuffles of the small inputs)."""
    x = np.ascontiguousarray(np.asarray(x), dtype=np.float32)
    blocks = np.ascontiguousarray(np.asarray(blocks), dtype=np.float32)
    # xt[(ni*16+q), c*32+b] = x[b, 128c + 16ni + q], then reverse c (c'=63-c)
    xt = x.T.reshape(64, 128, 32).transpose(1, 0, 2)[:, ::-1, :].reshape(128, 2048)
    xt = np.ascontiguousarray(xt)
    u = np.arange(W)
    ni = np.arange(8)
    split = dt_name == "split16"
    if not split:
        wt_name, mv_name = _split_dt(dt_name)
        np_w, np_m = _np_dtype(wt_name), _np_dtype(mv_name)
        xt_c = np.ascontiguousarray(xt.astype(np_m))
    else:
        xth, xtl = _hi_lo16(xt)
        xth, xtl = np.ascontiguousarray(xth), np.ascontiguousarray(xtl)
    in_maps = []
    for k in range(NCORES):
        m0 = k * MBLK
        idx = (m0 + u[None, :] - ni[:, None]) % NB        # [8, W]
        bigq = blocks[idx]                                 # [8, W, p, q]
        bigq = bigq.transpose(0, 3, 1, 2).reshape(128, W * 16)  # [(ni,q), (u,p)]
        if split:
            bqh, bql = _hi_lo16(bigq)
            in_maps.append({
                "xth": xth, "xtl": xtl,
                "bqh": np.ascontiguousarray(bqh), "bql": np.ascontiguousarray(bql),
            })
        else:
            in_maps.append(
                {"xt": xt_c, "bigq": np.ascontiguousarray(bigq.astype(np_w))}
            )
    return in_maps


def _assemble(results):
    y = np.empty((B, NB * 16), dtype=np.float32)
    for k in range(NCORES):
        o = np.asarray(results[k]["out"])  # [128 (mi,p), 256 (t',b)], t = 7-t'
        y[:, 1024 * k: 1024 * (k + 1)] = (
            o.reshape(128, 8, 32)[:, ::-1, :].transpose(2, 1, 0).reshape(32, 1024)
        )
    return y


def kernel(x, blocks):
    global _last_results
    from concourse.bass_utils import run_bass_kernel_spmd

    nc = _get_program(DTYPE)
    in_maps = _prep_inputs(x, blocks, DTYPE)
    res = run_bass_kernel_spmd(nc, in_maps, list(range(NCORES)))
    _last_results = res
    return _assemble(res.results)
